# revision 16
# baseline (speedup 1.0000x reference)
"""DispLoss kernel for Trainium2 (8 NeuronCores, Bass/Tile) -- v2.

Math notes
----------
reference computes, per pixel p (B*H*W of them):
    target = w_idx - disp
    mask   = valid & (disp < 192)
    pos    = clip(target + 0.1*W, 0, 1.1*W) / (1.1*W/255)      in [0, 255]
    lb = floor(pos); hb = min(lb+1,255); wh = pos-lb
    logp   = log_softmax(logits[:, :, p], axis=channels)
    ce     = -( (1-wh)*logp[lb] + wh*logp[hb] )
    logits_loss = sum(ce*mask)/msum;  coord_loss = sum(|coord-target|*mask)/msum

Device identities:
 *  ce*mask summed = sum mask*lse - sum_c hat(pos-c)*x[c]
    with hat(d) = relu(1-|d|) = -(min(|d|,1) - 1), so
    sum hat*x = -sum (min(|pos-c|,1)-1)*x[c]   (u-form, no sum-x needed)
 *  masked-out pixels get pos := -10 => u == 0 for all c => net 0.
 *  pos is affine in the pixel's W column with disp spanning <~37 bins, so
    with pixels ordered W-major each CH-pixel chunk's hat support lies in
    ONE 128-channel half (a few chunks straddle both) -> apass/stt run on
    one half per chunk, not two.

Layout: channels on partitions (2 halves of 128), pixels on the free axis,
pixel order (w, b, h) per core.  Per-pixel sumexp: es = exp(x0)+exp(x1)
(128 partitions) is column-summed by PE with a ones[128,1] stationary into
per-chunk PSUM rows lse_ps[rpc*k + j, :] -- no per-pixel restack, no
stationary reloads.  pos broadcast: 2 accumulating bf16 matmuls (integer +
fraction rows) -> exact fp32 pos in PSUM at bf16 moving rate.

Per core the device returns 5 scalars:
    [ sum u*x,  0,  sum mask*lse,  sum mask,  sum |coord-target|*mask ]
host combines: logits_loss = (masklse + sum_u_x)/msum.
"""

import os
import sys
from contextlib import ExitStack

import numpy as np

for _p in ("/opt/trn_rl_repo", "/root/.axon_site/_ro/trn_rl_repo"):
    if os.path.isdir(_p) and _p not in sys.path:
        sys.path.insert(0, _p)

B, H, W = 2, 384, 1216
NBINS = 256
NCORES = 8

# Device tiling configuration (full problem, per core).
CFG = dict(B=B, NB=NBINS, HC=H // NCORES, W=W, CH=3072, WIN=1024, LW=1536)

# chunk-half counter % MOD < RES -> DVE-form apass (clip identity, no abs
# needed: u = clip(d,0,1)+clip(-d,0,1)-1 = v1 - w1 - 1 with v1/w1 clips of
# pos), else ACT-form (Abs + min/add u-pass).  DVE-form costs ~3x the DVE
# time of an ACT-form half, so only a fraction moves over.
DVE_FORM_RES = 0
DVE_FORM_MOD = 11


def derived(cfg):
    PB = cfg["B"] * cfg["HC"] * cfg["W"]
    CH, WIN, LW = cfg["CH"], cfg["WIN"], cfg["LW"]
    NK = PB // CH
    NW = CH // WIN
    RPC = CH // LW
    assert NK * CH == PB, (NK, CH, PB)
    assert NW * WIN == CH and RPC * LW == CH
    assert WIN % 512 == 0 and LW % 512 == 0
    R = NK * RPC
    # PE cannot write psum partitions 96..127 (quadrant 3); keep rows < 96
    assert R <= 96, R
    return PB, NK, NW, RPC, R


def build_program(cfg, halves, dve_form=(DVE_FORM_RES, DVE_FORM_MOD)):
    import concourse.bacc as bacc
    import concourse.tile as tile
    from concourse import mybir

    AF = mybir.ActivationFunctionType
    OP = mybir.AluOpType
    f32 = mybir.dt.float32
    bf16 = mybir.dt.bfloat16

    NB = cfg["NB"]
    PB, NK, NW, RPC, R = derived(cfg)
    CH, WIN, LW = cfg["CH"], cfg["WIN"], cfg["LW"]
    assert len(halves) == NK

    nc = bacc.Bacc("TRN2", target_bir_lowering=False)
    xl = nc.dram_tensor("xl", [NB, PB], f32, kind="ExternalInput")
    phd = nc.dram_tensor("phd", [NK, CH], bf16, kind="ExternalInput")
    pld = nc.dram_tensor("pld", [NK, CH], bf16, kind="ExternalInput")
    maskd = nc.dram_tensor("maskd", [R, LW], f32, kind="ExternalInput")
    l1d = nc.dram_tensor("l1d", [R, LW], f32, kind="ExternalInput")
    cneg = nc.dram_tensor("cneg", [2, 128, 1], f32, kind="ExternalInput")
    cpos = nc.dram_tensor("cpos", [2, 128, 1], f32, kind="ExternalInput")
    cpos1 = nc.dram_tensor("cpos1", [2, 128, 1], f32, kind="ExternalInput")
    cneg1 = nc.dram_tensor("cneg1", [2, 128, 1], f32, kind="ExternalInput")
    outp = nc.dram_tensor("outp", [1, 5], f32, kind="ExternalOutput")

    NWIN = cfg["CH"] // cfg["WIN"]
    n_acc = 0
    ci = 0
    forms = []   # per (k) dict: half -> 'dve'|'act'
    for hs in halves:
        fk = {}
        for h in hs:
            if (ci * dve_form[0]) % dve_form[1] < dve_form[0]:
                fk[h] = "dve"
                n_acc += 2 * NWIN
            else:
                fk[h] = "act"
                n_acc += 1
            ci += 1
        forms.append(fk)

    with ExitStack() as ctx:
        tc = ctx.enter_context(tile.TileContext(nc))
        consts = ctx.enter_context(tc.tile_pool(name="consts", bufs=1))
        ppool = ctx.enter_context(tc.tile_pool(name="ppool", bufs=3))
        xpool = ctx.enter_context(tc.tile_pool(name="xpool", bufs=3))
        epool = ctx.enter_context(tc.tile_pool(name="epool", bufs=4))
        espool = ctx.enter_context(tc.tile_pool(name="espool", bufs=2))
        apool = ctx.enter_context(tc.tile_pool(name="apool", bufs=3))
        upool = ctx.enter_context(tc.tile_pool(name="upool", bufs=2))
        vpool = ctx.enter_context(tc.tile_pool(name="vpool", bufs=2))
        ypool = ctx.enter_context(tc.tile_pool(name="ypool", bufs=2))
        pospool = ctx.enter_context(tc.tile_pool(name="pospool", bufs=2,
                                                 space="PSUM"))
        lsepool = ctx.enter_context(tc.tile_pool(name="lsepool", bufs=1,
                                                 space="PSUM"))
        scrpool = ctx.enter_context(tc.tile_pool(name="scrpool", bufs=1,
                                                 space="PSUM"))
        smalls = ctx.enter_context(tc.tile_pool(name="smalls", bufs=1))

        ones_row = consts.tile([1, 128], bf16)
        nc.vector.memset(ones_row, 1.0)
        ones_f = consts.tile([128, 1], f32)
        nc.vector.memset(ones_f, 1.0)
        # [128, 63] zeros with an all-ones column at index 31: slicing
        # [:, 31-r:63-r] gives a [128,32] stationary whose one-hot column r
        # routes a column-sum into psum partition (32-aligned base + r).
        ohbig = consts.tile([128, 63], bf16)
        nc.vector.memset(ohbig, 0.0)
        nc.vector.memset(ohbig[:, 31:32], 1.0)

        ccn, ccp, ccp1, ccn1 = [], [], [], []
        for h in range(2):
            t1 = consts.tile([128, 1], f32, name=f"ccn{h}", tag=f"ccn{h}")
            nc.sync.dma_start(out=t1, in_=cneg[h])
            ccn.append(t1)
            t2 = consts.tile([128, 1], f32, name=f"ccp{h}", tag=f"ccp{h}")
            nc.sync.dma_start(out=t2, in_=cpos[h])
            ccp.append(t2)
            t3 = consts.tile([128, 1], f32, name=f"ccp1{h}", tag=f"ccp1{h}")
            nc.sync.dma_start(out=t3, in_=cpos1[h])
            ccp1.append(t3)
            t4 = consts.tile([128, 1], f32, name=f"ccn1{h}", tag=f"ccn1{h}")
            nc.sync.dma_start(out=t4, in_=cneg1[h])
            ccn1.append(t4)
        maskt = consts.tile([R, LW], f32)
        nc.sync.dma_start(out=maskt, in_=maskd[:, :])
        l1t = consts.tile([R, LW], f32)
        nc.sync.dma_start(out=l1t, in_=l1d[:, :])

        # psum tile holding per-pixel sumexp; zeroed once, every matmul
        # adds (start=False) -- one-hot stationary columns route each
        # chunk's sums to its row, zeros elsewhere.
        lse_ps = lsepool.tile([128, LW], f32)
        nc.vector.memset(lse_ps, 0.0)
        scr_ps = scrpool.tile([128, 8], f32)
        accs = smalls.tile([128, max(n_acc, 1)], f32)
        finals = smalls.tile([128, 5], f32)
        nc.vector.memset(finals, 0.0)

        # Walrus rejects self-loading matmuls with >1 sync wait.  These
        # dummy matmuls make PE observe the DVE-memset constants up front
        # so no later matmul needs a DVE wait for them.
        nc.tensor.matmul(out=scr_ps[:, 0:1], lhsT=ones_row,
                         rhs=ones_row[0:1, 0:1], start=True, stop=True)
        nc.tensor.matmul(out=scr_ps[0:32, 0:1], lhsT=ohbig[:, 31:63],
                         rhs=ohbig[:, 31:32], start=True, stop=True)
        nc.tensor.matmul(out=scr_ps[0:1, 1:2], lhsT=ones_f,
                         rhs=ones_f[:, 0:1], start=True, stop=True)

        ai = 0     # stt accumulator column index
        for k in range(NK):
            pth = ppool.tile([1, CH], bf16, tag="pth")
            nc.sync.dma_start(out=pth, in_=phd[k])
            ptl = ppool.tile([1, CH], bf16, tag="ptl")
            nc.sync.dma_start(out=ptl, in_=pld[k])
            # one SWDGE cast-DMA per chunk moving BOTH 128-channel halves
            # (3-dim access pattern) -- halves the per-dma_start overhead on
            # the 99%-busy SWDGE queue vs two 1.5MB transfers.
            xb = xpool.tile([128, 2 * CH], bf16, tag="xt")
            src = xl.rearrange("(j p) q -> p j q", j=2)[
                :, :, CH * k:CH * (k + 1)]
            nc.gpsimd.dma_start(out=xb.rearrange("p (j q) -> p j q", j=2),
                                in_=src)
            xts, ets = [], []
            for h in range(2):
                xt = xb[:, CH * h:CH * (h + 1)]
                xts.append(xt)
                et = epool.tile([128, CH], bf16, tag="et")
                nc.scalar.activation(out=et, in_=xt, func=AF.Exp)
                ets.append(et)
            es = espool.tile([128, CH], bf16, tag="es")
            nc.vector.tensor_tensor(out=es, in0=ets[0], in1=ets[1],
                                    op=OP.add)

            # pos broadcast windows + apass
            ats = {h: apool.tile([128, CH], bf16, tag=f"at{h}",
                                 name=f"at{h}")
                   for h in halves[k] if forms[k][h] == "act"}
            for wI in range(NW):
                w0 = wI * WIN
                pos_ps = pospool.tile([128, WIN], f32, tag="pos")
                # tiny pre-writer matmul absorbs the psum-slot WAR wait
                # so the real broadcast matmuls carry only the DMA wait
                nc.tensor.matmul(out=pos_ps[:, 0:1], lhsT=ones_row,
                                 rhs=ones_row[0:1, 0:1], start=True,
                                 stop=True)
                # psum bank limit: each matmul write must stay in one
                # 2KB bank -> split the broadcast at 512-fp32 boundaries
                for q0 in range(0, WIN, 512):
                    nc.tensor.matmul(out=pos_ps[:, q0:q0 + 512],
                                     lhsT=ones_row,
                                     rhs=pth[0:1, w0 + q0:w0 + q0 + 512],
                                     start=True, stop=False)
                    nc.tensor.matmul(out=pos_ps[:, q0:q0 + 512],
                                     lhsT=ones_row,
                                     rhs=ptl[0:1, w0 + q0:w0 + q0 + 512],
                                     start=False, stop=True)
                for h in halves[k]:
                    if forms[k][h] == "act":
                        nc.scalar.activation(out=ats[h][:, w0:w0 + WIN],
                                             in_=pos_ps, func=AF.Abs,
                                             bias=ccn[h], scale=1.0)
                    else:
                        # clip identity: u = v1 - w1 - 1 with
                        # v1 = clip(pos, c, c+1), w1 = clip(pos, c-1, c);
                        # accumulate (v1-1)*x and -w1*x per window.
                        v1 = vpool.tile([128, WIN], f32, tag="v1")
                        nc.vector.tensor_scalar(
                            v1, pos_ps, ccp1[h], ccp[h], OP.min, OP.max)
                        w1 = vpool.tile([128, WIN], f32, tag="w1")
                        nc.vector.tensor_scalar(
                            w1, pos_ps, ccn1[h], ccp[h], OP.max, OP.min)
                        yw = ypool.tile([128, WIN], bf16, tag="yw")
                        nc.vector.scalar_tensor_tensor(
                            out=yw, in0=v1, scalar=-1.0,
                            in1=xts[h][:, w0:w0 + WIN],
                            op0=OP.add, op1=OP.mult,
                            accum_out=accs[:, ai:ai + 1])
                        ai += 1
                        yw2 = ypool.tile([128, WIN], bf16, tag="yw")
                        nc.vector.scalar_tensor_tensor(
                            out=yw2, in0=w1, scalar=-1.0,
                            in1=xts[h][:, w0:w0 + WIN],
                            op0=OP.mult, op1=OP.mult,
                            accum_out=accs[:, ai:ai + 1])
                        ai += 1

            # sumexp column sums: one-hot stationary column routes each
            # (chunk, 512-column group)'s sums into its psum row; other
            # rows in the 32-row window receive +0.
            for j in range(CH // 512):
                q = j * 512
                row = RPC * k + q // LW
                col = q % LW
                b32 = 32 * (row // 32)
                r32 = row % 32
                nc.tensor.matmul(
                    out=lse_ps[b32:b32 + 32, col:col + 512],
                    lhsT=ohbig[:, 31 - r32:63 - r32],
                    rhs=es[:, q:q + 512],
                    start=False, stop=True, skip_group_check=True)

            # u = min(|pos-c|,1) - 1  (= -hat), then accumulate u*x
            for h in halves[k]:
                if forms[k][h] != "act":
                    continue
                ut = upool.tile([128, CH], bf16, tag="ut")
                nc.vector.tensor_scalar(ut, ats[h], 1.0, -1.0,
                                        OP.min, OP.add)
                yt = ypool.tile([128, CH], bf16, tag="yt")
                nc.vector.scalar_tensor_tensor(
                    out=yt, in0=ut, scalar=1.0, in1=xts[h],
                    op0=OP.mult, op1=OP.mult,
                    accum_out=accs[:, ai:ai + 1])
                ai += 1

        assert ai == n_acc, (ai, n_acc)
        # epilogue: lse, masked sums, final partition reduction
        lse_sb = smalls.tile([R, LW], f32)
        nc.scalar.activation(out=lse_sb, in_=lse_ps[0:R, :], func=AF.Ln)
        scr = smalls.tile([R, LW], bf16)
        nc.vector.scalar_tensor_tensor(
            out=scr, in0=lse_sb, scalar=1.0, in1=maskt,
            op0=OP.mult, op1=OP.mult, accum_out=finals[0:R, 2:3])
        scr2 = smalls.tile([R, LW], bf16)
        nc.vector.tensor_scalar(scr2, maskt, 1.0, None, OP.mult, OP.add,
                                accum_out=finals[0:R, 3:4])
        scr3 = smalls.tile([R, LW], bf16)
        nc.vector.tensor_scalar(scr3, l1t, 1.0, None, OP.mult, OP.add,
                                accum_out=finals[0:R, 4:5])
        nc.vector.tensor_reduce(finals[:, 0:1], accs,
                                axis=mybir.AxisListType.X, op=OP.add)
        nc.tensor.matmul(out=scr_ps[0:1, 2:7], lhsT=ones_f,
                         rhs=finals[:, 0:5], start=True, stop=True)
        out_sb = smalls.tile([1, 5], f32)
        nc.scalar.activation(out=out_sb, in_=scr_ps[0:1, 2:7],
                             func=AF.Copy)
        nc.sync.dma_start(out=outp[:, :], in_=out_sb)

    nc.compile()
    return nc


def _pos_mask_l1(cfg, coord, disp, valid):
    """Full-array pos/mask/l1 in the reference's math (fp32)."""
    Wc = cfg["W"]
    coord = np.asarray(coord, np.float32)
    disp = np.asarray(disp, np.float32)
    valid = np.asarray(valid, bool)
    wcol = np.arange(Wc, dtype=np.float32)
    target = (wcol[None, None, :] - disp).astype(np.float32)
    mask = (valid & (disp < np.float32(192.0))).astype(np.float32)
    labels = np.clip(target + np.float32(0.1 * Wc), np.float32(0.0),
                     np.float32(1.1 * Wc)).astype(np.float32)
    interval = np.float32(1.1 * Wc / 255.0)
    pos = (labels / interval).astype(np.float32)
    posm = np.where(mask > 0, pos, np.float32(-10.0)).astype(np.float32)
    l1m = (np.abs(coord - target) * mask).astype(np.float32)
    return posm, mask, l1m


def host_prep(cfg, coord, coord_logits, disp, valid, n_cores):
    """Slice + preprocess inputs per core.  Returns (in_maps, halves)."""
    import ml_dtypes

    Bc, NB, HC = cfg["B"], cfg["NB"], cfg["HC"]
    PB, NK, NW, RPC, R = derived(cfg)
    CH, LW = cfg["CH"], cfg["LW"]

    coord_logits = np.asarray(coord_logits, np.float32)
    posm, mask, l1m = _pos_mask_l1(cfg, coord, disp, valid)

    bf = ml_dtypes.bfloat16
    in_maps = []
    pos_cores = []
    for c in range(n_cores):
        r0, r1 = c * HC, (c + 1) * HC
        # pixel order (w, b, h)
        xl_c = np.ascontiguousarray(
            coord_logits[:, :, r0:r1, :].transpose(1, 3, 0, 2)
        ).reshape(NB, PB)
        pos_c = np.ascontiguousarray(
            posm[:, r0:r1, :].transpose(2, 0, 1)).reshape(PB)
        mask_c = np.ascontiguousarray(
            mask[:, r0:r1, :].transpose(2, 0, 1)).reshape(R, LW)
        l1_c = np.ascontiguousarray(
            l1m[:, r0:r1, :].transpose(2, 0, 1)).reshape(R, LW)
        ph = np.floor(pos_c + np.float32(0.5)).astype(np.float32)
        pl = (pos_c - ph).astype(np.float32)
        cvals = np.arange(256, dtype=np.float32).reshape(2, 128, 1)
        in_maps.append(dict(
            xl=xl_c,
            phd=ph.reshape(NK, CH).astype(bf),
            pld=pl.reshape(NK, CH).astype(bf),
            maskd=mask_c, l1d=l1_c,
            cneg=-cvals, cpos=cvals, cpos1=cvals + 1.0, cneg1=cvals - 1.0))
        pos_cores.append(pos_c)
    halves = compute_halves(cfg, pos_cores)
    return in_maps, halves


def compute_halves(cfg, pos_cores):
    """Per chunk: which 128-channel halves contain any hat support."""
    PB, NK, NW, RPC, R = derived(cfg)
    CH = cfg["CH"]
    pos = np.stack(pos_cores).reshape(len(pos_cores), NK, CH)
    halves = []
    for k in range(NK):
        p = pos[:, k, :]
        live = p > -5.0
        if not live.any():
            halves.append((0,))
            continue
        lo = max(0.0, float(p[live].min()) - 1.0)
        hi = min(255.0, float(p[live].max()) + 1.0)
        hs = tuple(h for h in (0, 1)
                   if lo < 128 * (h + 1) and hi >= 128 * h)
        halves.append(hs)
    return tuple(halves)


def combine(partials):
    """partials: list of (1,5) or (5,) arrays -> (objective, coord, logits)."""
    tot = np.sum([np.asarray(p, np.float64).reshape(5) for p in partials],
                 axis=0)
    uacc, _, masklse, msum, l1 = tot
    msum = msum + 1e-6
    coord_loss = l1 / msum
    logits_loss = (masklse + uacc) / msum
    objective = 0.1 * coord_loss + logits_loss
    return (np.float32(objective), np.float32(coord_loss),
            np.float32(logits_loss))


_prog_cache = {}


def _get_program(cfg, halves, dve_form=(DVE_FORM_RES, DVE_FORM_MOD)):
    k = (tuple(sorted(cfg.items())), halves, dve_form)
    if k not in _prog_cache:
        _prog_cache[k] = build_program(cfg, halves, dve_form=dve_form)
    return _prog_cache[k]


def kernel(coord, coord_logits, disp, valid):
    from concourse.bass_utils import run_bass_kernel_spmd

    in_maps, halves = host_prep(CFG, coord, coord_logits, disp, valid,
                                NCORES)
    nc = _get_program(CFG, halves)
    res = run_bass_kernel_spmd(nc, in_maps, core_ids=list(range(NCORES)))
    partials = [r["outp"] for r in res.results]
    return combine(partials)


# ---------------------------------------------------------------------------
# numpy model of the device program (for validation in test harnesses)
def model_partials(cfg, in_map, halves):
    """Emulate one core's device math in numpy (fp32-ish)."""
    NB = cfg["NB"]
    PB, NK, NW, RPC, R = derived(cfg)
    CH = cfg["CH"]
    xl = in_map["xl"].astype(np.float32)        # (NB, PB)
    pos = (in_map["phd"].astype(np.float32)
           + in_map["pld"].astype(np.float32)).reshape(NK, CH)
    uacc = 0.0
    for k in range(NK):
        xk = xl[:, k * CH:(k + 1) * CH]
        for h in halves[k]:
            cs = np.arange(128 * h, 128 * h + 128,
                           dtype=np.float32)[:, None]
            d = np.abs(pos[k][None, :] - cs)
            u = np.minimum(d, 1.0) - 1.0
            uacc += float((u * xk[128 * h:128 * h + 128]).sum(
                dtype=np.float64))
    lse = np.log(np.exp(xl).sum(axis=0, dtype=np.float64)).reshape(R,
                                                                   cfg["LW"])
    masklse = float((in_map["maskd"] * lse).sum(dtype=np.float64))
    msum = float(in_map["maskd"].sum(dtype=np.float64))
    l1 = float(in_map["l1d"].sum(dtype=np.float64))
    return np.array([uacc, 0.0, masklse, msum, l1], np.float64).reshape(1, 5)
